# revision 1
# baseline (speedup 1.0000x reference)
"""Trainium2 Bass kernel for nn_ChannelMixing (RWKV-style channel mixing).

Math: the reference's FFT decay-conv is the first-order IIR
    h[t] = mix*h[t-1] + x[t],  h[-1] = last_x/(1-mix)
and x_mix = (1-mix)*h, so with weights pre-scaled by (1-mix):
    k = h_k @ (Wk*(1-mix_k)).T,  r = h_r @ (Wr*(1-mix_r)).T
    out = sigmoid(r) * (relu(k)^2 @ Wv.T)

Sharding: time dimension L=4096 split over 8 cores (512 rows each) with a
64-step halo to warm up the scan state (decay <= sigmoid(1) ~ 0.731, so
carry across 64 steps < 3e-9 — below fp32 noise). Core 0 gets the exact
initial state via a per-core init column; no collectives.

Layout: everything [channel(P), time(F)]. The scan runs on the vector
engine (tensor_tensor_scan), the three 2048x2048 matmuls on the PE in
fp32r, activations on ACT, gating on DVE.
"""
import numpy as np
from contextlib import ExitStack

import concourse.bass as bass
from concourse import bacc
import concourse.tile as tile
import concourse.mybir as mybir
from concourse.bass_utils import run_bass_kernel_spmd

LEN, DIM = 4096, 2048
NCORES = 8
P = 128
HALO = 64

f32 = mybir.dt.float32
f32r = mybir.dt.float32r
Alu = mybir.AluOpType
Act = mybir.ActivationFunctionType

_cache = {}


def _build(dim, tloc, halo):
    """Build + compile the per-core SPMD program."""
    nt = dim // P          # channel tiles
    ts = tloc + halo       # time slab incl. halo
    ng = max(1, (dim // P) // 4)   # output m-groups of 4 m-tiles
    NF = 512 if tloc >= 512 else tloc   # matmul moving size (time)
    assert tloc % NF == 0
    nf = tloc // NF        # time blocks per matmul (1 at full size)

    nc = bacc.Bacc(trn_type="TRN2", debug=False)

    xs_d = nc.dram_tensor("xs", [dim, ts], f32, kind="ExternalInput").ap()
    dec_d = nc.dram_tensor("dec", [P, 2 * nt], f32, kind="ExternalInput").ap()  # SBUF image
    ini_d = nc.dram_tensor("ini", [P, 2 * nt], f32, kind="ExternalInput").ap()
    wk_d = nc.dram_tensor("wk", [dim, dim], f32r, kind="ExternalInput").ap()  # [d, i] pre-scaled
    wr_d = nc.dram_tensor("wr", [dim, dim], f32r, kind="ExternalInput").ap()
    wv_d = nc.dram_tensor("wv", [dim, dim], f32r, kind="ExternalInput").ap()  # [i, o]
    out_d = nc.dram_tensor("out", [dim, tloc], f32, kind="ExternalOutput").ap()

    with tile.TileContext(nc) as tc, ExitStack() as ctx:
        const = ctx.enter_context(tc.tile_pool(name="const", bufs=1))
        xs_pool = ctx.enter_context(tc.tile_pool(name="xs", bufs=6))
        h_pool = ctx.enter_context(tc.tile_pool(name="h", bufs=1))
        w_pool = ctx.enter_context(tc.tile_pool(name="w", bufs=12))
        ev_pool = ctx.enter_context(tc.tile_pool(name="ev", bufs=1))
        sc_pool = ctx.enter_context(tc.tile_pool(name="sc", bufs=3))
        o_pool = ctx.enter_context(tc.tile_pool(name="o", bufs=3))
        ps_pool = ctx.enter_context(tc.tile_pool(name="ps", bufs=2, space="PSUM"))

        # per-channel constants: [P, nt] tiles (col ct = chan tile ct)
        dec_t = const.tile([P, 2 * nt], f32)
        nc.scalar.dma_start(dec_t[:], dec_d)
        ini_t = const.tile([P, 2 * nt], f32)
        nc.scalar.dma_start(ini_t[:], ini_d)

        # ---- stage A: decay scans -> h_k, h_r in [chan, time] ----
        h = {"k": [None] * nt, "r": [None] * nt}
        for pi, p in enumerate(("k", "r")):
            for ct in range(nt):
                xs = xs_pool.tile([P, ts], f32, tag="xs", name=f"xs{p}{ct}")
                nc.scalar.dma_start(xs[:], xs_d[ct * P:(ct + 1) * P, :])
                dcol = dec_t[:, 2 * ct + pi: 2 * ct + pi + 1]
                # single scan over halo+body; core0's initial state is
                # h0*mix^-halo (host-prepped) so it decays to exactly h0
                # across the zero halo columns.
                hs = h_pool.tile([P, ts], f32r, tag=f"h{p}{ct}", name=f"hs{p}{ct}")
                nc.vector.tensor_tensor_scan(
                    hs[:], dcol.broadcast_to([P, ts]), xs[:],
                    ini_t[:, 2 * ct + pi: 2 * ct + pi + 1],
                    op0=Alu.mult, op1=Alu.add)
                h[p][ct] = hs[:, halo:]

        # ---- stage B helper: out[o_tile, t] = sum_kt w[kt,o].T @ rhs[kt] ----
        def big_matmul(w_dram, rhs_tiles, evict_fn, wtag):
            for g in range(ng):
                m4 = min(4, nt - 4 * g)
                psums = [ps_pool.tile([P, NF], f32, tag=f"ps{m}",
                                      name=f"ps_{wtag}_{g}_{m}") for m in range(m4)]
                for tb in range(nf):
                    for kt in range(nt):
                        wt = w_pool.tile([P, m4 * P], f32r, tag="w",
                                         name=f"wt_{wtag}_{g}_{kt}")
                        nc.sync.dma_start(
                            wt[:], w_dram[kt * P:(kt + 1) * P,
                                          g * 4 * P: g * 4 * P + m4 * P])
                        for m in range(m4):
                            nc.tensor.matmul(
                                psums[m][:], wt[:, m * P:(m + 1) * P],
                                rhs_tiles[kt][:, tb * NF:(tb + 1) * NF],
                                start=(kt == 0), stop=(kt == nt - 1))
                    for m in range(m4):
                        evict_fn(g * 4 + m, tb, psums[m])

        # k path: evict = relu then square -> sq tiles (f32r)
        sq = [ev_pool.tile([P, tloc], f32r, tag=f"sq{i}", name=f"sq{i}") for i in range(nt)]

        def evict_k(mi, tb, psum):
            rr = sc_pool.tile([P, NF], f32, tag="rr")
            nc.scalar.activation(rr[:], psum[:], Act.Relu)
            nc.vector.tensor_mul(sq[mi][:, tb * NF:(tb + 1) * NF], rr[:], rr[:])

        # r path: evict = sigmoid -> sig tiles (f32)
        sig = [ev_pool.tile([P, tloc], f32, tag=f"sg{i}", name=f"sg{i}") for i in range(nt)]

        def evict_r(mi, tb, psum):
            nc.scalar.activation(sig[mi][:, tb * NF:(tb + 1) * NF], psum[:], Act.Sigmoid)

        # v path: evict = gate with sigmoid(r) -> DMA out
        def evict_v(mi, tb, psum):
            ot = o_pool.tile([P, NF], f32, tag="ot")
            nc.vector.tensor_mul(ot[:], psum[:], sig[mi][:, tb * NF:(tb + 1) * NF])
            nc.sync.dma_start(out_d[mi * P:(mi + 1) * P, tb * NF:(tb + 1) * NF], ot[:])

        # PE warmup during the scan phase: keeps HAM at K=8/8 so the real
        # matmul stream starts warm. Uses the first weight tile as both
        # operands; results are discarded (psum slot reused with start=True).
        wsz = min(NF, dim)
        wm = min(P, wsz)
        wt0 = w_pool.tile([P, wsz], f32r, tag="w", name="wt_warm")
        nc.sync.dma_start(wt0[:], wk_d[0:P, 0:wsz])
        ps_w = ps_pool.tile([P, wsz], f32, tag="ps0", name="ps_warm")
        for _ in range(28):
            nc.tensor.matmul(ps_w[0:wm, :], wt0[:, 0:wm],
                             wt0[:], start=True, stop=True)

        big_matmul(wk_d, h["k"], evict_k, "wk")
        big_matmul(wr_d, h["r"], evict_r, "wr")
        big_matmul(wv_d, sq, evict_v, "wv")

    nc.compile()
    return nc


def _sigmoid(v):
    return 1.0 / (1.0 + np.exp(-v.astype(np.float64)))


def _prep(x, Wk, Wr, Wv, mix_k, mix_r, lxk, lxr, ncores, halo):
    """Host-side prep: transposes, weight pre-scaling, per-core slabs."""
    dim = x.shape[1]
    tloc = x.shape[0] // ncores
    mk = _sigmoid(mix_k).astype(np.float32)
    mr = _sigmoid(mix_r).astype(np.float32)
    h0k = (lxk / (1.0 - mk)).astype(np.float32)
    h0r = (lxr / (1.0 - mr)).astype(np.float32)
    P = 128
    nt = dim // P
    dec = np.empty((P, 2 * nt), np.float32)   # SBUF image: [p, 2*ct+path]
    dec[:, 0::2] = mk.reshape(nt, P).T
    dec[:, 1::2] = mr.reshape(nt, P).T

    wk = np.ascontiguousarray((Wk * (1.0 - mk)[None, :]).T.astype(np.float32))
    wr = np.ascontiguousarray((Wr * (1.0 - mr)[None, :]).T.astype(np.float32))
    wv = np.ascontiguousarray(Wv.T.astype(np.float32))

    xT = np.ascontiguousarray(x.T.astype(np.float32))       # [dim, L]
    in_maps = []
    for c in range(ncores):
        t0 = c * tloc
        slab = np.empty((dim, halo + tloc), np.float32)
        if c == 0:
            slab[:, :halo] = 0.0
            bk = (h0k.astype(np.float64) * (1.0 / mk.astype(np.float64)) ** halo
                  ).astype(np.float32)
            br = (h0r.astype(np.float64) * (1.0 / mr.astype(np.float64)) ** halo
                  ).astype(np.float32)
            ini = np.empty((P, 2 * nt), np.float32)
            ini[:, 0::2] = bk.reshape(nt, P).T
            ini[:, 1::2] = br.reshape(nt, P).T
        else:
            slab[:, :halo] = xT[:, t0 - halo: t0]
            ini = np.zeros((P, 2 * nt), np.float32)
        slab[:, halo:] = xT[:, t0: t0 + tloc]
        in_maps.append({
            "xs": slab, "dec": dec, "ini": np.ascontiguousarray(ini),
            "wk": wk, "wr": wr, "wv": wv,
        })
    return in_maps


def kernel(x, Wk, Wr, Wv, mix_k, mix_r, last_x_mix_k, last_x_mix_r):
    x = np.asarray(x, np.float32)
    Wk = np.asarray(Wk, np.float32)
    Wr = np.asarray(Wr, np.float32)
    Wv = np.asarray(Wv, np.float32)
    mix_k = np.asarray(mix_k, np.float32)
    mix_r = np.asarray(mix_r, np.float32)
    lxk = np.asarray(last_x_mix_k, np.float32)
    lxr = np.asarray(last_x_mix_r, np.float32)

    L, dim = x.shape
    tloc = L // NCORES
    key = (dim, tloc, HALO)
    if key not in _cache:
        _cache[key] = _build(dim, tloc, HALO)
    nc = _cache[key]

    in_maps = _prep(x, Wk, Wr, Wv, mix_k, mix_r, lxk, lxr, NCORES, HALO)
    # First execution on a cold device occasionally returns
    # NRT_EXEC_UNIT_UNRECOVERABLE; a retry has always succeeded.
    res = None
    for attempt in range(3):
        try:
            res = run_bass_kernel_spmd(nc, in_maps, core_ids=list(range(NCORES)))
            break
        except Exception:
            if attempt == 2:
                raise

    out = np.empty((L, dim), np.float32)
    for c in range(NCORES):
        out[c * tloc:(c + 1) * tloc, :] = res.results[c]["out"].T
    return out



# revision 8
# speedup vs baseline: 1.1494x; 1.1494x over previous
"""Trainium2 Bass kernel for nn_ChannelMixing (RWKV-style channel mixing).

Math: the reference's FFT decay-conv is the first-order IIR
    h[t] = mix*h[t-1] + x[t],  h[-1] = last_x/(1-mix)
and x_mix = (1-mix)*h, so with weights pre-scaled by (1-mix):
    k = h_k @ (Wk*(1-mix_k)).T,  r = h_r @ (Wr*(1-mix_r)).T
    out = sigmoid(r) * (relu(k)^2 @ Wv.T)

Sharding: time dimension L=4096 split over 8 cores (512 rows each) with a
64-step halo to warm up the scan state (decay <= sigmoid(1) ~ 0.731, so
carry across 64 steps < 3e-9 — below fp32 noise). Core 0 gets the exact
initial state via a per-core init column; no collectives.

Layout: everything [channel(P), time(F)]. The k-path scans run on the
vector engine, r-path scans on gpsimd, the three 2048x2048 matmuls on the
PE in bf16 (fp32 psum). Weights are shipped bf16 in a tile-major layout so
each DMA is one contiguous 512KB block covering 4 contraction tiles.
"""
import numpy as np
from contextlib import ExitStack

import concourse.bass as bass
from concourse import bacc
import concourse.tile as tile
import concourse.mybir as mybir
from concourse.bass_utils import run_bass_kernel_spmd

LEN, DIM = 4096, 2048
NCORES = 8
P = 128
HALO = 64

f32 = mybir.dt.float32
bf16 = mybir.dt.bfloat16
Alu = mybir.AluOpType
Act = mybir.ActivationFunctionType

_cache = {}


def _build(dim, tloc, halo):
    """Build + compile the per-core SPMD program."""
    nt = dim // P          # channel tiles (16)
    ts = tloc + halo       # time slab incl. halo (576)
    ng = nt // 4           # output m-groups of 4 m-tiles (4)
    NF = tloc              # matmul moving size (512)
    KB = 4                 # kt tiles per weight DMA block

    nc = bacc.Bacc(trn_type="TRN2", debug=False)

    xs_d = nc.dram_tensor("xs", [dim, ts], f32, kind="ExternalInput").ap()
    dec_d = nc.dram_tensor("dec", [P, 2 * nt], f32, kind="ExternalInput").ap()
    ini_d = nc.dram_tensor("ini", [P, 2 * nt], f32, kind="ExternalInput").ap()
    # tile-major bf16 weights: row block (g*KB+b)*P holds kt=4b..4b+3 for
    # out-group g, cols [j*NF+c] = tile (kt=4b+j)[:, c]
    wk_d = nc.dram_tensor("wk", [dim, KB * NF], bf16, kind="ExternalInput").ap()
    wr_d = nc.dram_tensor("wr", [dim, KB * NF], bf16, kind="ExternalInput").ap()
    wv_d = nc.dram_tensor("wv", [dim, KB * NF], bf16, kind="ExternalInput").ap()
    out_d = nc.dram_tensor("out", [dim, tloc], f32, kind="ExternalOutput").ap()

    with tile.TileContext(nc) as tc, ExitStack() as ctx:
        const = ctx.enter_context(tc.tile_pool(name="const", bufs=1))
        xs_pool = ctx.enter_context(tc.tile_pool(name="xs", bufs=1))
        h_pool = ctx.enter_context(tc.tile_pool(name="h", bufs=1))
        w_pool = ctx.enter_context(tc.tile_pool(name="w", bufs=6))
        ev_pool = ctx.enter_context(tc.tile_pool(name="ev", bufs=1))
        sc_pool = ctx.enter_context(tc.tile_pool(name="sc", bufs=3))
        o_pool = ctx.enter_context(tc.tile_pool(name="o", bufs=3))
        ps_pool = ctx.enter_context(tc.tile_pool(name="ps", bufs=1, space="PSUM"))

        # per-channel constants: [P, nt] tiles (col ct = chan tile ct)
        dec_t = const.tile([P, 2 * nt], f32)
        nc.scalar.dma_start(dec_t[:], dec_d)
        ini_t = const.tile([P, 2 * nt], f32)
        nc.scalar.dma_start(ini_t[:], ini_d)

        # ---- stage A: x slabs (loaded once), decay scans -> h_k (DVE),
        #      h_r (gpsimd), both in [chan, time] bf16 ----
        xs = []
        for ct in range(nt):
            t = xs_pool.tile([P, ts], f32, tag=f"xs{ct}", name=f"xs{ct}")
            nc.scalar.dma_start(t[:], xs_d[ct * P:(ct + 1) * P, :])
            xs.append(t)

        h = {"k": [None] * nt, "r": [None] * nt}
        for pi, (p, eng) in enumerate((("k", nc.vector), ("r", nc.vector))):
            for ct in range(nt):
                dcol = dec_t[:, 2 * ct + pi: 2 * ct + pi + 1]
                hs = h_pool.tile([P, ts], bf16, tag=f"h{p}{ct}", name=f"hs{p}{ct}")
                eng.tensor_tensor_scan(
                    hs[:], dcol.broadcast_to([P, ts]), xs[ct][:],
                    ini_t[:, 2 * ct + pi: 2 * ct + pi + 1],
                    op0=Alu.mult, op1=Alu.add)
                h[p][ct] = hs[:, halo:]

        # ---- stage B helper: out[o_tile, t] = sum_kt w[kt,o].T @ rhs[kt] ----
        def big_matmul(w_dram, rhs_tiles, evict_fn, wtag, junk=False):
            for g in range(ng):
                # 8 PSUM banks as 8 explicit tags: group g uses banks
                # 4*(g%2)..4*(g%2)+3 so consecutive groups double-buffer
                psums = [ps_pool.tile([P, NF], f32, tag=f"ps{4 * (g % 2) + m}",
                                      name=f"ps_{wtag}_{g}_{m}") for m in range(4)]
                jps = None
                if junk and g == 0:
                    # filler targets next group's (idle) banks to keep the
                    # PE busy while the scan stream paces real work
                    jps = [ps_pool.tile([P, NF], f32, tag=f"ps{4 + m}",
                                        name=f"ps_{wtag}_j_{m}") for m in range(2)]
                for b in range(nt // KB):
                    wt = w_pool.tile([P, KB * NF], bf16, tag="w",
                                     name=f"wt_{wtag}_{g}_{b}")
                    nc.sync.dma_start(
                        wt[:], w_dram[(g * (nt // KB) + b) * P:
                                      (g * (nt // KB) + b + 1) * P, :])
                    for j in range(KB):
                        kt = b * KB + j
                        for m in range(4):
                            nc.tensor.matmul(
                                psums[m][:],
                                wt[:, j * NF + m * P: j * NF + (m + 1) * P],
                                rhs_tiles[kt][:, 0:NF],
                                start=(kt == 0), stop=(kt == nt - 1))
                        if jps is not None and 3 <= kt < nt - 1:
                            for m in range(2):
                                nc.tensor.matmul(
                                    jps[m][:],
                                    wt[:, j * NF + m * P: j * NF + (m + 1) * P],
                                    rhs_tiles[kt][:, 0:NF], start=True, stop=True)
                for m in range(4):
                    evict_fn(g * 4 + m, psums[m])

        # k path: evict = relu then square, both on ACT (DVE is busy with
        # the r-path scans through the whole Wk phase) -> sq tiles (bf16)
        sq = [ev_pool.tile([P, NF], bf16, tag=f"sq{i}", name=f"sq{i}")
              for i in range(nt)]

        def evict_k(mi, psum):
            rr = sc_pool.tile([P, NF], f32, tag="rr")
            nc.scalar.activation(rr[:], psum[:], Act.Relu)
            nc.scalar.activation(sq[mi][:], rr[:], Act.Square)

        # r path: evict = sigmoid (ACT) -> sig tiles (f32)
        sig = [ev_pool.tile([P, NF], f32, tag=f"sg{i}", name=f"sg{i}")
               for i in range(nt)]

        def evict_r(mi, psum):
            nc.scalar.activation(sig[mi][:], psum[:], Act.Sigmoid)

        # v path: evict = gate with sigmoid(r) (DVE) -> DMA out (scalar q)
        def evict_v(mi, psum):
            ot = o_pool.tile([P, NF], f32, tag="ot")
            nc.vector.tensor_mul(ot[:], psum[:], sig[mi][:])
            nc.scalar.dma_start(out_d[mi * P:(mi + 1) * P, :], ot[:])

        big_matmul(wk_d, h["k"], evict_k, "wk", junk=True)
        big_matmul(wr_d, h["r"], evict_r, "wr")
        big_matmul(wv_d, sq, evict_v, "wv")

    nc.compile()
    return nc


def _sigmoid(v):
    return 1.0 / (1.0 + np.exp(-v.astype(np.float64)))


def _tile_major(wT, nt, ng, P, NF, KB):
    """[dim, dim] -> tile-major [ng*(nt/KB)*P, KB*NF] bf16 image.

    Row block (g*(nt/KB)+b)*P, col j*NF+c = wT[(KB*b+j)*P+p, g*NF+c].
    """
    import ml_dtypes
    dim = wT.shape[0]
    A = wT.reshape(nt // KB, KB, P, ng, NF).transpose(3, 0, 2, 1, 4)
    return np.ascontiguousarray(A.reshape(dim, KB * NF).astype(ml_dtypes.bfloat16))


def _prep(x, Wk, Wr, Wv, mix_k, mix_r, lxk, lxr, ncores, halo):
    """Host-side prep: transposes, weight pre-scaling, per-core slabs."""
    dim = x.shape[1]
    tloc = x.shape[0] // ncores
    mk = _sigmoid(mix_k).astype(np.float32)
    mr = _sigmoid(mix_r).astype(np.float32)
    h0k = (lxk / (1.0 - mk)).astype(np.float32)
    h0r = (lxr / (1.0 - mr)).astype(np.float32)
    P = 128
    nt = dim // P
    ng = nt // 4
    NF = tloc
    KB = 4
    dec = np.empty((P, 2 * nt), np.float32)   # SBUF image: [p, 2*ct+path]
    dec[:, 0::2] = mk.reshape(nt, P).T
    dec[:, 1::2] = mr.reshape(nt, P).T

    wk = _tile_major((Wk * (1.0 - mk)[None, :]).T.astype(np.float32),
                     nt, ng, P, NF, KB)
    wr = _tile_major((Wr * (1.0 - mr)[None, :]).T.astype(np.float32),
                     nt, ng, P, NF, KB)
    wv = _tile_major(Wv.T.astype(np.float32), nt, ng, P, NF, KB)

    xT = np.ascontiguousarray(x.T.astype(np.float32))       # [dim, L]
    in_maps = []
    for c in range(ncores):
        t0 = c * tloc
        slab = np.empty((dim, halo + tloc), np.float32)
        if c == 0:
            slab[:, :halo] = 0.0
            bk = (h0k.astype(np.float64) * (1.0 / mk.astype(np.float64)) ** halo
                  ).astype(np.float32)
            br = (h0r.astype(np.float64) * (1.0 / mr.astype(np.float64)) ** halo
                  ).astype(np.float32)
            ini = np.empty((P, 2 * nt), np.float32)
            ini[:, 0::2] = bk.reshape(nt, P).T
            ini[:, 1::2] = br.reshape(nt, P).T
        else:
            slab[:, :halo] = xT[:, t0 - halo: t0]
            ini = np.zeros((P, 2 * nt), np.float32)
        slab[:, halo:] = xT[:, t0: t0 + tloc]
        in_maps.append({
            "xs": slab, "dec": dec, "ini": np.ascontiguousarray(ini),
            "wk": wk, "wr": wr, "wv": wv,
        })
    return in_maps


def kernel(x, Wk, Wr, Wv, mix_k, mix_r, last_x_mix_k, last_x_mix_r):
    x = np.asarray(x, np.float32)
    Wk = np.asarray(Wk, np.float32)
    Wr = np.asarray(Wr, np.float32)
    Wv = np.asarray(Wv, np.float32)
    mix_k = np.asarray(mix_k, np.float32)
    mix_r = np.asarray(mix_r, np.float32)
    lxk = np.asarray(last_x_mix_k, np.float32)
    lxr = np.asarray(last_x_mix_r, np.float32)

    L, dim = x.shape
    tloc = L // NCORES
    key = (dim, tloc, HALO)
    if key not in _cache:
        _cache[key] = _build(dim, tloc, HALO)
    nc = _cache[key]

    in_maps = _prep(x, Wk, Wr, Wv, mix_k, mix_r, lxk, lxr, NCORES, HALO)
    # First execution on a cold device occasionally returns
    # NRT_EXEC_UNIT_UNRECOVERABLE; a retry has always succeeded.
    res = None
    for attempt in range(3):
        try:
            res = run_bass_kernel_spmd(nc, in_maps, core_ids=list(range(NCORES)))
            break
        except Exception:
            if attempt == 2:
                raise

    out = np.empty((L, dim), np.float32)
    for c in range(NCORES):
        out[c * tloc:(c + 1) * tloc, :] = res.results[c]["out"].T
    return out


# revision 9
# speedup vs baseline: 1.1668x; 1.0152x over previous
"""Trainium2 Bass kernel for nn_ChannelMixing (RWKV-style channel mixing).

Math: the reference's FFT decay-conv is the first-order IIR
    h[t] = mix*h[t-1] + x[t],  h[-1] = last_x/(1-mix)
and x_mix = (1-mix)*h, so with weights pre-scaled by (1-mix):
    k = h_k @ (Wk*(1-mix_k)).T,  r = h_r @ (Wr*(1-mix_r)).T
    out = sigmoid(r) * (relu(k)^2 @ Wv.T)

Sharding: time dimension L=4096 split over 8 cores (512 rows each) with a
64-step halo to warm up the scan state (decay <= sigmoid(1) ~ 0.731, so
carry across 64 steps < 3e-9 — below fp32 noise). Core 0 gets the exact
initial state via a per-core init column; no collectives.

Layout: everything [channel(P), time(F)]. Decay scans on the vector
engine (fp32 state, bf16 out), the three 2048x2048 matmuls on the PE in
bf16 (fp32 psum, FWL weight loads), activations on ACT, gating on DVE.
Weights ship bf16 tile-major so each DMA is one contiguous 512KB block of
4 contraction tiles; the per-channel decay/init constants ride as 4 extra
columns of each x slab so no tiny straggler DMA gates the first scan.
"""
import numpy as np
from contextlib import ExitStack

import concourse.bass as bass
from concourse import bacc
import concourse.tile as tile
import concourse.mybir as mybir
from concourse.bass_utils import run_bass_kernel_spmd

LEN, DIM = 4096, 2048
NCORES = 8
P = 128
HALO = 64

f32 = mybir.dt.float32
bf16 = mybir.dt.bfloat16
Alu = mybir.AluOpType
Act = mybir.ActivationFunctionType

_cache = {}


def _build(dim, tloc, halo):
    """Build + compile the per-core SPMD program."""
    nt = dim // P          # channel tiles (16)
    ts = tloc + halo       # time slab incl. halo (576)
    ng = nt // 4           # output m-groups of 4 m-tiles (4)
    NF = tloc              # matmul moving size (512)
    KB = 4                 # kt tiles per weight DMA block
    XC = 4                 # extra xs columns: dec_k, dec_r, ini_k, ini_r

    nc = bacc.Bacc(trn_type="TRN2", debug=False)

    xs_d = nc.dram_tensor("xs", [dim, ts + XC], f32, kind="ExternalInput").ap()
    # tile-major bf16 weights: row block (g*KB+b)*P holds kt=4b..4b+3 for
    # out-group g, cols [j*NF+c] = tile (kt=4b+j)[:, c]
    wk_d = nc.dram_tensor("wk", [dim, KB * NF], bf16, kind="ExternalInput").ap()
    wr_d = nc.dram_tensor("wr", [dim, KB * NF], bf16, kind="ExternalInput").ap()
    wv_d = nc.dram_tensor("wv", [dim, KB * NF], bf16, kind="ExternalInput").ap()
    out_d = nc.dram_tensor("out", [dim, tloc], f32, kind="ExternalOutput").ap()

    with tile.TileContext(nc) as tc, ExitStack() as ctx:
        xs_pool = ctx.enter_context(tc.tile_pool(name="xs", bufs=1))
        h_pool = ctx.enter_context(tc.tile_pool(name="h", bufs=1))
        w_pool = ctx.enter_context(tc.tile_pool(name="w", bufs=6))
        ev_pool = ctx.enter_context(tc.tile_pool(name="ev", bufs=1))
        sc_pool = ctx.enter_context(tc.tile_pool(name="sc", bufs=3))
        o_pool = ctx.enter_context(tc.tile_pool(name="o", bufs=3))
        ps_pool = ctx.enter_context(tc.tile_pool(name="ps", bufs=1, space="PSUM"))

        # ---- stage A: x slabs (one DMA each, consts embedded), decay
        #      scans -> h_k / h_r in [chan, time] bf16 on DVE ----
        xs = []
        for ct in range(nt):
            t = xs_pool.tile([P, ts + XC], f32, tag=f"xs{ct}", name=f"xs{ct}")
            nc.scalar.dma_start(t[:], xs_d[ct * P:(ct + 1) * P, :])
            xs.append(t)

        h = {"k": [None] * nt, "r": [None] * nt}
        for pi, p in enumerate(("k", "r")):
            for ct in range(nt):
                dcol = xs[ct][:, ts + pi: ts + pi + 1]
                icol = xs[ct][:, ts + 2 + pi: ts + 3 + pi]
                hs = h_pool.tile([P, ts], bf16, tag=f"h{p}{ct}", name=f"hs{p}{ct}")
                nc.vector.tensor_tensor_scan(
                    hs[:], dcol.broadcast_to([P, ts]), xs[ct][:, 0:ts],
                    icol, op0=Alu.mult, op1=Alu.add)
                h[p][ct] = hs[:, halo:]

        # ---- stage B helper: out[o_tile, t] = sum_kt w[kt,o].T @ rhs[kt] ----
        def big_matmul(w_dram, rhs_fn, evict_fn, wtag):
            for g in range(ng):
                # 8 PSUM banks as 8 explicit tags: group g uses banks
                # 4*(g%2)..4*(g%2)+3 so consecutive groups double-buffer
                psums = [ps_pool.tile([P, NF], f32, tag=f"ps{4 * (g % 2) + m}",
                                      name=f"ps_{wtag}_{g}_{m}") for m in range(4)]
                for b in range(nt // KB):
                    wt = w_pool.tile([P, KB * NF], bf16, tag="w",
                                     name=f"wt_{wtag}_{g}_{b}")
                    nc.sync.dma_start(
                        wt[:], w_dram[(g * (nt // KB) + b) * P:
                                      (g * (nt // KB) + b + 1) * P, :])
                    for j in range(KB):
                        kt = b * KB + j
                        for m in range(4):
                            nc.tensor.matmul(
                                psums[m][:],
                                wt[:, j * NF + m * P: j * NF + (m + 1) * P],
                                rhs_fn(kt),
                                start=(kt == 0), stop=(kt == nt - 1))
                for m in range(4):
                    evict_fn(g * 4 + m, psums[m])

        # k path: evict = relu then square, both on ACT (DVE is busy with
        # the r-path scans through the whole Wk phase) -> sq slices (bf16)
        sq = ev_pool.tile([P, nt * NF], bf16, tag="sq", name="sq")

        def evict_k(mi, psum):
            rr = sc_pool.tile([P, NF], f32, tag="rr")
            nc.scalar.activation(rr[:], psum[:], Act.Relu)
            nc.scalar.activation(sq[:, mi * NF:(mi + 1) * NF], rr[:], Act.Square)

        # r path: evict = sigmoid (ACT) -> sig slices (f32)
        sig = ev_pool.tile([P, nt * NF], f32, tag="sg", name="sig")

        def evict_r(mi, psum):
            nc.scalar.activation(sig[:, mi * NF:(mi + 1) * NF], psum[:],
                                 Act.Sigmoid)

        # v path: evict = gate with sigmoid(r) (DVE) -> DMA out (scalar q)
        def evict_v(mi, psum):
            ot = o_pool.tile([P, NF], f32, tag="ot")
            nc.vector.tensor_mul(ot[:], psum[:], sig[:, mi * NF:(mi + 1) * NF])
            nc.scalar.dma_start(out_d[mi * P:(mi + 1) * P, :], ot[:])

        big_matmul(wk_d, lambda kt: h["k"][kt][:, 0:NF], evict_k, "wk")
        big_matmul(wr_d, lambda kt: h["r"][kt][:, 0:NF], evict_r, "wr")
        big_matmul(wv_d, lambda kt: sq[:, kt * NF:(kt + 1) * NF], evict_v, "wv")

    nc.compile()
    return nc


def _sigmoid(v):
    return 1.0 / (1.0 + np.exp(-v.astype(np.float64)))


def _tile_major(wT, nt, ng, P, NF, KB):
    """[dim, dim] -> tile-major [ng*(nt/KB)*P, KB*NF] bf16 image.

    Row block (g*(nt/KB)+b)*P, col j*NF+c = wT[(KB*b+j)*P+p, g*NF+c].
    """
    import ml_dtypes
    dim = wT.shape[0]
    A = wT.reshape(nt // KB, KB, P, ng, NF).transpose(3, 0, 2, 1, 4)
    return np.ascontiguousarray(A.reshape(dim, KB * NF).astype(ml_dtypes.bfloat16))


def _prep(x, Wk, Wr, Wv, mix_k, mix_r, lxk, lxr, ncores, halo):
    """Host-side prep: transposes, weight pre-scaling, per-core slabs."""
    dim = x.shape[1]
    tloc = x.shape[0] // ncores
    mk = _sigmoid(mix_k).astype(np.float32)
    mr = _sigmoid(mix_r).astype(np.float32)
    h0k = (lxk / (1.0 - mk)).astype(np.float32)
    h0r = (lxr / (1.0 - mr)).astype(np.float32)
    P = 128
    nt = dim // P
    ng = nt // 4
    NF = tloc
    KB = 4
    ts = tloc + halo

    wk = _tile_major((Wk * (1.0 - mk)[None, :]).T.astype(np.float32),
                     nt, ng, P, NF, KB)
    wr = _tile_major((Wr * (1.0 - mr)[None, :]).T.astype(np.float32),
                     nt, ng, P, NF, KB)
    wv = _tile_major(Wv.T.astype(np.float32), nt, ng, P, NF, KB)

    # core-0 exact initial state, pre-decayed across the zero halo
    bk = (h0k.astype(np.float64) * (1.0 / mk.astype(np.float64)) ** halo
          ).astype(np.float32)
    br = (h0r.astype(np.float64) * (1.0 / mr.astype(np.float64)) ** halo
          ).astype(np.float32)

    xT = np.ascontiguousarray(x.T.astype(np.float32))       # [dim, L]
    in_maps = []
    for c in range(ncores):
        t0 = c * tloc
        slab = np.empty((dim, ts + 4), np.float32)
        if c == 0:
            slab[:, :halo] = 0.0
            slab[:, ts + 2] = bk
            slab[:, ts + 3] = br
        else:
            slab[:, :halo] = xT[:, t0 - halo: t0]
            slab[:, ts + 2] = 0.0
            slab[:, ts + 3] = 0.0
        slab[:, halo:ts] = xT[:, t0: t0 + tloc]
        slab[:, ts + 0] = mk
        slab[:, ts + 1] = mr
        in_maps.append({"xs": slab, "wk": wk, "wr": wr, "wv": wv})
    return in_maps


def kernel(x, Wk, Wr, Wv, mix_k, mix_r, last_x_mix_k, last_x_mix_r):
    x = np.asarray(x, np.float32)
    Wk = np.asarray(Wk, np.float32)
    Wr = np.asarray(Wr, np.float32)
    Wv = np.asarray(Wv, np.float32)
    mix_k = np.asarray(mix_k, np.float32)
    mix_r = np.asarray(mix_r, np.float32)
    lxk = np.asarray(last_x_mix_k, np.float32)
    lxr = np.asarray(last_x_mix_r, np.float32)

    L, dim = x.shape
    tloc = L // NCORES
    key = (dim, tloc, HALO)
    if key not in _cache:
        _cache[key] = _build(dim, tloc, HALO)
    nc = _cache[key]

    in_maps = _prep(x, Wk, Wr, Wv, mix_k, mix_r, lxk, lxr, NCORES, HALO)
    # First execution on a cold device occasionally returns
    # NRT_EXEC_UNIT_UNRECOVERABLE; a retry has always succeeded.
    res = None
    for attempt in range(3):
        try:
            res = run_bass_kernel_spmd(nc, in_maps, core_ids=list(range(NCORES)))
            break
        except Exception:
            if attempt == 2:
                raise

    out = np.empty((L, dim), np.float32)
    for c in range(NCORES):
        out[c * tloc:(c + 1) * tloc, :] = res.results[c]["out"].T
    return out


# revision 12
# speedup vs baseline: 1.1956x; 1.0247x over previous
"""Trainium2 Bass kernel for nn_ChannelMixing (RWKV-style channel mixing).

Math: the reference's FFT decay-conv is the first-order IIR
    h[t] = mix*h[t-1] + x[t],  h[-1] = last_x/(1-mix)
and x_mix = (1-mix)*h, so with weights pre-scaled by (1-mix):
    k = h_k @ (Wk*(1-mix_k)).T,  r = h_r @ (Wr*(1-mix_r)).T
    out = sigmoid(r) * (relu(k)^2 @ Wv.T)

Sharding: time dimension L=4096 split over 8 cores (512 rows each) with a
64-step halo to warm up the scan state (decay <= sigmoid(1) ~ 0.731, so
carry across 64 steps < 3e-9 — below fp32 noise). Core 0 gets the exact
initial state via a per-core init column; no collectives.

Layout: everything [channel(P), time(F)]. Decay scans on the vector
engine (fp32 state, bf16 out), the three 2048x2048 matmuls on the PE in
bf16 (fp32 psum, FWL weight loads), activations on ACT, gating on DVE.
Weights ship bf16 tile-major so each DMA is one contiguous 512KB block of
4 contraction tiles; the per-channel decay/init constants ride as 4 extra
columns of each x slab so no tiny straggler DMA gates the first scan.
"""
import numpy as np
from contextlib import ExitStack

import concourse.bass as bass
from concourse import bacc
import concourse.tile as tile
import concourse.mybir as mybir
from concourse.bass_utils import run_bass_kernel_spmd

LEN, DIM = 4096, 2048
NCORES = 8
P = 128
HALO = 16
KB = 2      # kt tiles per weight DMA block

f32 = mybir.dt.float32
bf16 = mybir.dt.bfloat16
Alu = mybir.AluOpType
Act = mybir.ActivationFunctionType

_cache = {}


def _build(dim, tloc, halo):
    """Build + compile the per-core SPMD program."""
    nt = dim // P          # channel tiles (16)
    ts = tloc + halo       # time slab incl. halo (576)
    ng = nt // 4           # output m-groups of 4 m-tiles (4)
    NF = tloc              # matmul moving size (512)
    XC = 4                 # extra xs columns: dec_k, dec_r, ini_k, ini_r

    nc = bacc.Bacc(trn_type="TRN2", debug=False)

    xs_d = nc.dram_tensor("xs", [dim, ts + XC], f32, kind="ExternalInput").ap()
    # tile-major bf16 weights: row block (g*KB+b)*P holds kt=4b..4b+3 for
    # out-group g, cols [j*NF+c] = tile (kt=4b+j)[:, c]
    wrows = ng * (nt // KB) * P
    wk_d = nc.dram_tensor("wk", [wrows, KB * NF], bf16, kind="ExternalInput").ap()
    wr_d = nc.dram_tensor("wr", [wrows, KB * NF], bf16, kind="ExternalInput").ap()
    wv_d = nc.dram_tensor("wv", [wrows, KB * NF], bf16, kind="ExternalInput").ap()
    out_d = nc.dram_tensor("out", [dim, tloc], f32, kind="ExternalOutput").ap()

    with tile.TileContext(nc) as tc, ExitStack() as ctx:
        xs_pool = ctx.enter_context(tc.tile_pool(name="xs", bufs=1))
        h_pool = ctx.enter_context(tc.tile_pool(name="h", bufs=1))
        w_pool = ctx.enter_context(tc.tile_pool(name="w", bufs=8))
        ev_pool = ctx.enter_context(tc.tile_pool(name="ev", bufs=1))
        sc_pool = ctx.enter_context(tc.tile_pool(name="sc", bufs=3))
        o_pool = ctx.enter_context(tc.tile_pool(name="o", bufs=3))
        ps_pool = ctx.enter_context(tc.tile_pool(name="ps", bufs=1, space="PSUM"))

        # ---- stage A: x slabs (one DMA each, consts embedded), decay
        #      scans -> h_k / h_r in [chan, time] bf16 on DVE ----
        xs = []
        for ct in range(nt):
            t = xs_pool.tile([P, ts + XC], f32, tag=f"xs{ct}", name=f"xs{ct}")
            nc.scalar.dma_start(t[:], xs_d[ct * P:(ct + 1) * P, :])
            xs.append(t)

        h = {"k": [None] * nt, "r": [None] * nt}

        def scans(pi, p):
            for ct in range(nt):
                dcol = xs[ct][:, ts + pi: ts + pi + 1]
                icol = xs[ct][:, ts + 2 + pi: ts + 3 + pi]
                hs = h_pool.tile([P, ts], bf16, tag=f"h{p}{ct}", name=f"hs{p}{ct}")
                nc.vector.tensor_tensor_scan(
                    hs[:], dcol.broadcast_to([P, ts]), xs[ct][:, 0:ts],
                    icol, op0=Alu.mult, op1=Alu.add)
                h[p][ct] = hs[:, halo:]

        # ---- stage B helper: out[o_tile, t] = sum_kt w[kt,o].T @ rhs[kt] ----
        def big_matmul(w_dram, rhs_fn, evict_fn, wtag):
            for g in range(ng):
                # 8 PSUM banks as 8 explicit tags: group g uses banks
                # 4*(g%2)..4*(g%2)+3 so consecutive groups double-buffer
                psums = [ps_pool.tile([P, NF], f32, tag=f"ps{4 * (g % 2) + m}",
                                      name=f"ps_{wtag}_{g}_{m}") for m in range(4)]
                for b in range(nt // KB):
                    wt = w_pool.tile([P, KB * NF], bf16, tag="w",
                                     name=f"wt_{wtag}_{g}_{b}")
                    nc.sync.dma_start(
                        wt[:], w_dram[(g * (nt // KB) + b) * P:
                                      (g * (nt // KB) + b + 1) * P, :])
                    for j in range(KB):
                        kt = b * KB + j
                        for m in range(4):
                            nc.tensor.matmul(
                                psums[m][:],
                                wt[:, j * NF + m * P: j * NF + (m + 1) * P],
                                rhs_fn(kt),
                                start=(kt == 0), stop=(kt == nt - 1))
                for m in range(4):
                    evict_fn(g * 4 + m, psums[m])

        # k path: evict = relu then square, both on ACT (DVE is busy with
        # the r-path scans through the whole Wk phase) -> sq slices (bf16)
        sq = ev_pool.tile([P, nt * NF], bf16, tag="sq", name="sq")

        def evict_k(mi, psum):
            rr = sc_pool.tile([P, NF], f32, tag="rr")
            nc.scalar.activation(rr[:], psum[:], Act.Relu)
            nc.scalar.activation(sq[:, mi * NF:(mi + 1) * NF], rr[:], Act.Square)

        # r path: evict = sigmoid (ACT) -> sig slices (f32)
        sig = ev_pool.tile([P, nt * NF], f32, tag="sg", name="sig")

        def evict_r(mi, psum):
            nc.scalar.activation(sig[:, mi * NF:(mi + 1) * NF], psum[:],
                                 Act.Sigmoid)

        # v path: evict = gate with sigmoid(r) (DVE) -> DMA out (scalar q)
        def evict_v(mi, psum):
            ot = o_pool.tile([P, NF], f32, tag="ot")
            nc.vector.tensor_mul(ot[:], psum[:], sig[:, mi * NF:(mi + 1) * NF])
            nc.scalar.dma_start(out_d[mi * P:(mi + 1) * P, :], ot[:])

        scans(0, "k")
        big_matmul(wk_d, lambda kt: h["k"][kt][:, 0:NF], evict_k, "wk")
        scans(1, "r")
        big_matmul(wr_d, lambda kt: h["r"][kt][:, 0:NF], evict_r, "wr")
        big_matmul(wv_d, lambda kt: sq[:, kt * NF:(kt + 1) * NF], evict_v, "wv")

    nc.compile()
    return nc


def _sigmoid(v):
    return 1.0 / (1.0 + np.exp(-v.astype(np.float64)))


def _tile_major(wT, nt, ng, P, NF, KB):
    """[dim, dim] -> tile-major [ng*(nt/KB)*P, KB*NF] bf16 image.

    Row block (g*(nt/KB)+b)*P, col j*NF+c = wT[(KB*b+j)*P+p, g*NF+c].
    """
    import ml_dtypes
    A = wT.reshape(nt // KB, KB, P, ng, NF).transpose(3, 0, 2, 1, 4)
    return np.ascontiguousarray(
        A.reshape(ng * (nt // KB) * P, KB * NF).astype(ml_dtypes.bfloat16))


def _prep(x, Wk, Wr, Wv, mix_k, mix_r, lxk, lxr, ncores, halo):
    """Host-side prep: transposes, weight pre-scaling, per-core slabs."""
    dim = x.shape[1]
    tloc = x.shape[0] // ncores
    mk = _sigmoid(mix_k).astype(np.float32)
    mr = _sigmoid(mix_r).astype(np.float32)
    h0k = (lxk / (1.0 - mk)).astype(np.float32)
    h0r = (lxr / (1.0 - mr)).astype(np.float32)
    P = 128
    nt = dim // P
    ng = nt // 4
    NF = tloc
    ts = tloc + halo

    wk = _tile_major((Wk * (1.0 - mk)[None, :]).T.astype(np.float32),
                     nt, ng, P, NF, KB)
    wr = _tile_major((Wr * (1.0 - mr)[None, :]).T.astype(np.float32),
                     nt, ng, P, NF, KB)
    wv = _tile_major(Wv.T.astype(np.float32), nt, ng, P, NF, KB)

    # core-0 exact initial state, pre-decayed across the zero halo
    bk = (h0k.astype(np.float64) * (1.0 / mk.astype(np.float64)) ** halo
          ).astype(np.float32)
    br = (h0r.astype(np.float64) * (1.0 / mr.astype(np.float64)) ** halo
          ).astype(np.float32)

    xT = np.ascontiguousarray(x.T.astype(np.float32))       # [dim, L]
    in_maps = []
    for c in range(ncores):
        t0 = c * tloc
        slab = np.empty((dim, ts + 4), np.float32)
        if c == 0:
            slab[:, :halo] = 0.0
            slab[:, ts + 2] = bk
            slab[:, ts + 3] = br
        else:
            slab[:, :halo] = xT[:, t0 - halo: t0]
            slab[:, ts + 2] = 0.0
            slab[:, ts + 3] = 0.0
        slab[:, halo:ts] = xT[:, t0: t0 + tloc]
        slab[:, ts + 0] = mk
        slab[:, ts + 1] = mr
        in_maps.append({"xs": slab, "wk": wk, "wr": wr, "wv": wv})
    return in_maps


def kernel(x, Wk, Wr, Wv, mix_k, mix_r, last_x_mix_k, last_x_mix_r):
    x = np.asarray(x, np.float32)
    Wk = np.asarray(Wk, np.float32)
    Wr = np.asarray(Wr, np.float32)
    Wv = np.asarray(Wv, np.float32)
    mix_k = np.asarray(mix_k, np.float32)
    mix_r = np.asarray(mix_r, np.float32)
    lxk = np.asarray(last_x_mix_k, np.float32)
    lxr = np.asarray(last_x_mix_r, np.float32)

    L, dim = x.shape
    tloc = L // NCORES
    key = (dim, tloc, HALO)
    if key not in _cache:
        _cache[key] = _build(dim, tloc, HALO)
    nc = _cache[key]

    in_maps = _prep(x, Wk, Wr, Wv, mix_k, mix_r, lxk, lxr, NCORES, HALO)
    # First execution on a cold device occasionally returns
    # NRT_EXEC_UNIT_UNRECOVERABLE; a retry has always succeeded.
    res = None
    for attempt in range(3):
        try:
            res = run_bass_kernel_spmd(nc, in_maps, core_ids=list(range(NCORES)))
            break
        except Exception:
            if attempt == 2:
                raise

    out = np.empty((L, dim), np.float32)
    for c in range(NCORES):
        out[c * tloc:(c + 1) * tloc, :] = res.results[c]["out"].T
    return out


# revision 18
# speedup vs baseline: 1.2312x; 1.0298x over previous
"""Trainium2 Bass kernel for nn_ChannelMixing (RWKV-style channel mixing).

Math: the reference's FFT decay-conv is the first-order IIR
    h[t] = mix*h[t-1] + x[t],  h[-1] = last_x/(1-mix)
and x_mix = (1-mix)*h, so with weights pre-scaled by (1-mix):
    k = h_k @ (Wk*(1-mix_k)).T,  r = h_r @ (Wr*(1-mix_r)).T
    out = sigmoid(r) * (relu(k)^2 @ Wv.T)

Sharding: time dimension L=4096 split over 8 cores (512 rows each) with a
16-step halo to warm up the scan state (decay <= sigmoid(1) ~ 0.731, so
carry across 16 steps < 7e-3 — below the bf16 noise floor). Core 0 gets
the exact initial state via a per-core init column; no collectives.

Layout: everything [channel(P), time(F)]. Decay scans on the vector
engine (fp32 state, bf16 in/out), the three 2048x2048 matmuls on the PE
in bf16 (fp32 psum, FWL weight loads), activations on ACT, gating on DVE.
Weights ship bf16 tile-major so each DMA is one contiguous 256KB block of
2 contraction tiles; the per-channel decay/init constants go out as one
small fp32 DMA issued first, while the DMA engines are still idle.
"""
import numpy as np
from contextlib import ExitStack

import concourse.bass as bass
from concourse import bacc
import concourse.tile as tile
import concourse.mybir as mybir
from concourse.bass_utils import run_bass_kernel_spmd

LEN, DIM = 4096, 2048
NCORES = 8
P = 128
HALO = 16
KB = 2      # kt tiles per weight DMA block

f32 = mybir.dt.float32
bf16 = mybir.dt.bfloat16
Alu = mybir.AluOpType
Act = mybir.ActivationFunctionType

_cache = {}


def _build(dim, tloc, halo):
    """Build + compile the per-core SPMD program."""
    nt = dim // P          # channel tiles (16)
    ts = tloc + halo       # time slab incl. halo (576)
    ng = nt // 4           # output m-groups of 4 m-tiles (4)
    NF = tloc              # matmul moving size (512)

    nc = bacc.Bacc(trn_type="TRN2", debug=False)

    xs_d = nc.dram_tensor("xs", [dim, ts], bf16, kind="ExternalInput").ap()
    cst_d = nc.dram_tensor("cst", [P, 4 * nt], f32, kind="ExternalInput").ap()
    # tile-major bf16 weights: row block (g*KB+b)*P holds kt=4b..4b+3 for
    # out-group g, cols [j*NF+c] = tile (kt=4b+j)[:, c]
    wrows = ng * (nt // KB) * P
    wk_d = nc.dram_tensor("wk", [wrows, KB * NF], bf16, kind="ExternalInput").ap()
    wr_d = nc.dram_tensor("wr", [wrows, KB * NF], bf16, kind="ExternalInput").ap()
    wv_d = nc.dram_tensor("wv", [wrows, KB * NF], bf16, kind="ExternalInput").ap()
    out_d = nc.dram_tensor("out", [dim, tloc], f32, kind="ExternalOutput").ap()

    with tile.TileContext(nc) as tc, ExitStack() as ctx:
        xs_pool = ctx.enter_context(tc.tile_pool(name="xs", bufs=1))
        h_pool = ctx.enter_context(tc.tile_pool(name="h", bufs=1))
        w_pool = ctx.enter_context(tc.tile_pool(name="w", bufs=8))
        ev_pool = ctx.enter_context(tc.tile_pool(name="ev", bufs=1))
        sc_pool = ctx.enter_context(tc.tile_pool(name="sc", bufs=3))
        o_pool = ctx.enter_context(tc.tile_pool(name="o", bufs=3))
        ps_pool = ctx.enter_context(tc.tile_pool(name="ps", bufs=1, space="PSUM"))

        # ---- stage A: decay/init consts (first DMA, engines idle), bf16
        #      x slabs, scans -> h_k / h_r in [chan, time] bf16 on DVE ----
        cst = xs_pool.tile([P, 4 * nt], f32, tag="cst", name="cst")
        nc.scalar.dma_start(cst[:], cst_d)
        xs = []
        for ct in range(nt):
            t = xs_pool.tile([P, ts], bf16, tag=f"xs{ct}", name=f"xs{ct}")
            nc.scalar.dma_start(t[:], xs_d[ct * P:(ct + 1) * P, :])
            xs.append(t)

        h = {"k": [None] * nt, "r": [None] * nt}

        def scans(pi, p):
            for ct in range(nt):
                dcol = cst[:, pi * nt + ct: pi * nt + ct + 1]
                icol = cst[:, (2 + pi) * nt + ct: (2 + pi) * nt + ct + 1]
                hs = h_pool.tile([P, ts], bf16, tag=f"h{p}{ct}", name=f"hs{p}{ct}")
                nc.vector.tensor_tensor_scan(
                    hs[:], dcol.broadcast_to([P, ts]), xs[ct][:, 0:ts],
                    icol, op0=Alu.mult, op1=Alu.add)
                h[p][ct] = hs[:, halo:]

        # ---- stage B helper: out[o_tile, t] = sum_kt w[kt,o].T @ rhs[kt] ----
        def big_matmul(w_dram, rhs_fn, evict_fn, wtag):
            for g in range(ng):
                # 8 PSUM banks as 8 explicit tags: group g uses banks
                # 4*(g%2)..4*(g%2)+3 so consecutive groups double-buffer
                psums = [ps_pool.tile([P, NF], f32, tag=f"ps{4 * (g % 2) + m}",
                                      name=f"ps_{wtag}_{g}_{m}") for m in range(4)]
                for b in range(nt // KB):
                    wt = w_pool.tile([P, KB * NF], bf16, tag="w",
                                     name=f"wt_{wtag}_{g}_{b}")
                    nc.sync.dma_start(
                        wt[:], w_dram[(g * (nt // KB) + b) * P:
                                      (g * (nt // KB) + b + 1) * P, :])
                    for j in range(KB):
                        kt = b * KB + j
                        for m in range(4):
                            nc.tensor.matmul(
                                psums[m][:],
                                wt[:, j * NF + m * P: j * NF + (m + 1) * P],
                                rhs_fn(kt),
                                start=(kt == 0), stop=(kt == nt - 1))
                for m in range(4):
                    evict_fn(g * 4 + m, psums[m])

        # k path: evict = relu then square, both on ACT (DVE is busy with
        # the r-path scans through the whole Wk phase) -> sq slices (bf16)
        sq = ev_pool.tile([P, nt * NF], bf16, tag="sq", name="sq")

        def evict_k(mi, psum):
            rr = sc_pool.tile([P, NF], f32, tag="rr")
            nc.scalar.activation(rr[:], psum[:], Act.Relu)
            nc.scalar.activation(sq[:, mi * NF:(mi + 1) * NF], rr[:], Act.Square)

        # r path: evict = sigmoid (ACT) -> sig slices (f32)
        sig = ev_pool.tile([P, nt * NF], f32, tag="sg", name="sig")

        def evict_r(mi, psum):
            nc.scalar.activation(sig[:, mi * NF:(mi + 1) * NF], psum[:],
                                 Act.Sigmoid)

        # v path: evict = gate with sigmoid(r) (DVE) -> DMA out, alternating
        # queues so the final drain isn't serialized on one issue stream
        def evict_v(mi, psum):
            ot = o_pool.tile([P, NF], f32, tag="ot")
            nc.vector.tensor_mul(ot[:], psum[:], sig[:, mi * NF:(mi + 1) * NF])
            q = nc.scalar if mi % 2 == 0 else nc.sync
            q.dma_start(out_d[mi * P:(mi + 1) * P, :], ot[:])

        scans(0, "k")
        big_matmul(wk_d, lambda kt: h["k"][kt][:, 0:NF], evict_k, "wk")
        scans(1, "r")
        big_matmul(wr_d, lambda kt: h["r"][kt][:, 0:NF], evict_r, "wr")
        big_matmul(wv_d, lambda kt: sq[:, kt * NF:(kt + 1) * NF], evict_v, "wv")

    nc.compile()
    return nc


def _sigmoid(v):
    return 1.0 / (1.0 + np.exp(-v.astype(np.float64)))


def _tile_major(wT, nt, ng, P, NF, KB):
    """[dim, dim] -> tile-major [ng*(nt/KB)*P, KB*NF] bf16 image.

    Row block (g*(nt/KB)+b)*P, col j*NF+c = wT[(KB*b+j)*P+p, g*NF+c].
    """
    import ml_dtypes
    A = wT.reshape(nt // KB, KB, P, ng, NF).transpose(3, 0, 2, 1, 4)
    return np.ascontiguousarray(
        A.reshape(ng * (nt // KB) * P, KB * NF).astype(ml_dtypes.bfloat16))


def _prep(x, Wk, Wr, Wv, mix_k, mix_r, lxk, lxr, ncores, halo):
    """Host-side prep: transposes, weight pre-scaling, per-core slabs."""
    dim = x.shape[1]
    tloc = x.shape[0] // ncores
    mk = _sigmoid(mix_k).astype(np.float32)
    mr = _sigmoid(mix_r).astype(np.float32)
    h0k = (lxk / (1.0 - mk)).astype(np.float32)
    h0r = (lxr / (1.0 - mr)).astype(np.float32)
    P = 128
    nt = dim // P
    ng = nt // 4
    NF = tloc
    ts = tloc + halo

    wk = _tile_major((Wk * (1.0 - mk)[None, :]).T.astype(np.float32),
                     nt, ng, P, NF, KB)
    wr = _tile_major((Wr * (1.0 - mr)[None, :]).T.astype(np.float32),
                     nt, ng, P, NF, KB)
    wv = _tile_major(Wv.T.astype(np.float32), nt, ng, P, NF, KB)

    # core-0 exact initial state, pre-decayed across the zero halo
    bk = (h0k.astype(np.float64) * (1.0 / mk.astype(np.float64)) ** halo
          ).astype(np.float32)
    br = (h0r.astype(np.float64) * (1.0 / mr.astype(np.float64)) ** halo
          ).astype(np.float32)

    import ml_dtypes
    xT = np.ascontiguousarray(x.T.astype(np.float32))       # [dim, L]
    in_maps = []
    for c in range(ncores):
        t0 = c * tloc
        slab = np.empty((dim, ts), np.float32)
        cst = np.zeros((128, 4 * nt), np.float32)
        cst[:, 0:nt] = mk.reshape(nt, 128).T
        cst[:, nt:2 * nt] = mr.reshape(nt, 128).T
        if c == 0:
            slab[:, :halo] = 0.0
            cst[:, 2 * nt:3 * nt] = bk.reshape(nt, 128).T
            cst[:, 3 * nt:4 * nt] = br.reshape(nt, 128).T
        else:
            slab[:, :halo] = xT[:, t0 - halo: t0]
        slab[:, halo:ts] = xT[:, t0: t0 + tloc]
        in_maps.append({"xs": np.ascontiguousarray(slab.astype(ml_dtypes.bfloat16)),
                        "cst": cst, "wk": wk, "wr": wr, "wv": wv})
    return in_maps


def kernel(x, Wk, Wr, Wv, mix_k, mix_r, last_x_mix_k, last_x_mix_r):
    x = np.asarray(x, np.float32)
    Wk = np.asarray(Wk, np.float32)
    Wr = np.asarray(Wr, np.float32)
    Wv = np.asarray(Wv, np.float32)
    mix_k = np.asarray(mix_k, np.float32)
    mix_r = np.asarray(mix_r, np.float32)
    lxk = np.asarray(last_x_mix_k, np.float32)
    lxr = np.asarray(last_x_mix_r, np.float32)

    L, dim = x.shape
    tloc = L // NCORES
    key = (dim, tloc, HALO)
    if key not in _cache:
        _cache[key] = _build(dim, tloc, HALO)
    nc = _cache[key]

    in_maps = _prep(x, Wk, Wr, Wv, mix_k, mix_r, lxk, lxr, NCORES, HALO)
    # First execution on a cold device occasionally returns
    # NRT_EXEC_UNIT_UNRECOVERABLE; a retry has always succeeded.
    res = None
    for attempt in range(3):
        try:
            res = run_bass_kernel_spmd(nc, in_maps, core_ids=list(range(NCORES)))
            break
        except Exception:
            if attempt == 2:
                raise

    out = np.empty((L, dim), np.float32)
    for c in range(NCORES):
        out[c * tloc:(c + 1) * tloc, :] = res.results[c]["out"].T
    return out
